# revision 30
# baseline (speedup 1.0000x reference)
"""Trainium2 Bass kernel for nn_DSSnetwork (DSS-GNN message passing).

Strategy (graph-level data parallelism, 8 graphs per core):
  - x kept SBUF-resident, feature-major [128, 20000] fp32, ping-pong across layers.
  - Per-subgraph aggregation as dense block-diagonal pair matmuls on PE
    (stationary = transposed node-major x pair [100,128] bf16, moving = pair
    adjacency [100,100] bf16 streamed from HBM; extra 50 identity columns
    accumulate the cross-subgraph-copy sum for the h2 branch).
  - Dense W matmuls in float32r (fp22 multiplies) at 1 cycle/row.
  - BatchNorm is global across all 160000 (resp 3200) rows: per-core
    (sum, sumsq) go through one 2KB AllReduce per layer; affine apply is
    folded into per-feature scale/bias vectors.
"""

import numpy as np
import ml_dtypes

# ---- static problem sizes (must match reference.setup_inputs) ----
G, S, N, D, DEG, L = 64, 50, 50, 128, 8, 4
TN = G * S * N
N_ORIG = G * N
N_SUBG = G * S
EPS = 1e-5

NC = 8                 # cores
GPC = G // NC          # graphs per core = 8
PN = GPC * S * N       # nodes per core = 20000
PO = GPC * N           # original nodes per core = 400
NPAIR = GPC * S // 2   # subgraph pairs per core = 200
CH = 500               # node chunk (5 pairs, 10 subgraphs)
NCH = PN // CH         # 40 chunks
GN = S * N             # nodes per graph = 2500
HG = GN // 2           # half graph = 1250
PNP = 20096            # PN padded to multiple of 128 (157 tiles)

BF16 = ml_dtypes.bfloat16

_BUILD_CACHE = {}


def _build_program(cc=True):
    import concourse.bass as bass
    import concourse.mybir as mybir
    import concourse.tile as tile
    import concourse.bacc as bacc

    f32 = mybir.dt.float32
    f32r = mybir.dt.float32r
    bf = mybir.dt.bfloat16
    AF = mybir.ActivationFunctionType
    OP = mybir.AluOpType

    nc_ = bacc.Bacc("TRN2", target_bir_lowering=False, debug=False, num_devices=NC)

    with tile.TileContext(nc_) as tc:
        nc = tc.nc

        xT_in = nc.dram_tensor("xT", [10, D, PN // 10], f32r, kind="ExternalInput")
        Ap_in = nc.dram_tensor("Apair", [NCH, 100, 750], bf, kind="ExternalInput")
        Ao_in = nc.dram_tensor("AoT", [N, PO], bf, kind="ExternalInput")
        Wr_in = nc.dram_tensor("Wr", [D, L * D], f32r, kind="ExternalInput")
        Wn_in = nc.dram_tensor("Wn", [D, L * D], f32r, kind="ExternalInput")
        Ws_in = nc.dram_tensor("Ws", [D, L * D], f32r, kind="ExternalInput")
        Wsn_in = nc.dram_tensor("Wsn", [D, L * D], f32r, kind="ExternalInput")
        vecs_in = nc.dram_tensor("vecs", [D, L * 6], f32, kind="ExternalInput")
        fW1_in = nc.dram_tensor("fW1", [D, 256], f32r, kind="ExternalInput")
        fW2_in = nc.dram_tensor("fW2", [D, 20], f32r, kind="ExternalInput")
        fb1_in = nc.dram_tensor("fb1", [D, 2], f32, kind="ExternalInput")
        fb2_in = nc.dram_tensor("fb2", [10, 1], f32, kind="ExternalInput")
        id_in = nc.dram_tensor("ident", [D, D], f32r, kind="ExternalInput")
        idb_in = nc.dram_tensor("identbf", [D, D], bf, kind="ExternalInput")
        Y_out = nc.dram_tensor("Y", [10, GPC], f32, kind="ExternalOutput")

        with (
            tc.tile_pool(name="persist", bufs=1) as PP,
            tc.tile_pool(name="xnmp", bufs=6) as XP,
            tc.tile_pool(name="aggp", bufs=3) as AGP,
            tc.tile_pool(name="app", bufs=3) as APP,
            tc.tile_pool(name="castp", bufs=5) as CSP,
            tc.tile_pool(name="sqp", bufs=2) as SQP,
            tc.tile_pool(name="ps", bufs=2, space="PSUM") as PS,
            tc.tile_pool(name="ps1", bufs=1, space="PSUM") as PS1,
            tc.tile_pool(name="dram", bufs=2, space="DRAM") as DP,
        ):
            xa = PP.tile([D, PN], f32r)
            xb = PP.tile([D, PN], f32r)
            Wr = PP.tile([D, L * D], f32r)
            Wn = PP.tile([D, L * D], f32r)
            Ws = PP.tile([D, L * D], f32r)
            Wsn = PP.tile([D, L * D], f32r)
            vecs = PP.tile([D, L * 6], f32)
            fW1 = PP.tile([D, 256], f32r)
            fW2 = PP.tile([D, 20], f32r)
            fb1 = PP.tile([D, 2], f32)
            fb2 = PP.tile([10, 1], f32)
            ident = PP.tile([D, D], f32r)
            identbf = PP.tile([D, D], bf)
            Ao_sb = PP.tile([N, PO], bf)
            accA = PP.tile([D, NCH], f32)
            accB = PP.tile([D, NCH], f32)
            accP = PP.tile([D, NCH], f32)
            hg = PP.tile([D, GPC], f32)
            stats = PP.tile([D, 4], f32)
            gstats = PP.tile([D, 4], f32)
            xsum_f = PP.tile([D, PO], f32r)
            xsum_b = PP.tile([D, PO], bf)
            xsum_nm = PP.tile([N, GPC * D], bf)
            aggo_f = PP.tile([D, PO], f32r)
            h2raw = PP.tile([D, PO], f32)
            H2 = PP.tile([D, PO], f32)
            sq2 = PP.tile([D, PO], f32)
            sc = PP.tile([D, 16], f32)
            r1 = PP.tile([D, 16], f32r)
            y_sb = PP.tile([10, GPC], f32)

            # ---- loads ----
            for i in range(10):
                nc.sync.dma_start(xa[:, i * 2000:(i + 1) * 2000],
                                  xT_in[i, :, :])
            nc.sync.dma_start(Wr[:], Wr_in[:])
            nc.sync.dma_start(Wn[:], Wn_in[:])
            nc.sync.dma_start(Ws[:], Ws_in[:])
            nc.sync.dma_start(Wsn[:], Wsn_in[:])
            nc.sync.dma_start(vecs[:], vecs_in[:])
            nc.sync.dma_start(fW1[:], fW1_in[:])
            nc.sync.dma_start(fW2[:], fW2_in[:])
            nc.sync.dma_start(fb1[:], fb1_in[:])
            nc.sync.dma_start(fb2[:], fb2_in[:])
            nc.sync.dma_start(ident[:], id_in[:])
            nc.sync.dma_start(identbf[:], idb_in[:])
            nc.sync.dma_start(Ao_sb[:], Ao_in[:])

            xbufs = [xa, xb]
            for l in range(L):
                xc = xbufs[l % 2]
                xn = xbufs[(l + 1) % 2]
                WrT = Wr[:, l * D:(l + 1) * D]
                WnT = Wn[:, l * D:(l + 1) * D]
                WsT = Ws[:, l * D:(l + 1) * D]
                WsnT = Wsn[:, l * D:(l + 1) * D]
                bng_c = vecs[:, l * 6 + 0:l * 6 + 1]
                bnb_c = vecs[:, l * 6 + 1:l * 6 + 2]
                bnsg_c = vecs[:, l * 6 + 2:l * 6 + 3]
                bnsb_c = vecs[:, l * 6 + 3:l * 6 + 4]
                b_c = vecs[:, l * 6 + 4:l * 6 + 5]
                bs_c = vecs[:, l * 6 + 5:l * 6 + 6]

                xsum_ps = PS1.tile([D, PO], f32, tag="xsum", name=f"xsum_ps_{l}")

                # --- transposes are emitted lazily, interleaved with chunks,
                #     so the PE stream pipelines instead of phase-serializing ---
                xnm_tiles = []

                def emit_group(grp):
                    tp = PS.tile([100, 512], f32r, tag="tr", name=f"tp_{l}_{grp}")
                    for j in range(4):
                        p = grp * 4 + j
                        nc.tensor.transpose(
                            tp[0:100, j * 128:(j + 1) * 128],
                            xc[:, p * 100:(p + 1) * 100],
                            ident[:],
                        )
                    xnm = XP.tile([100, 512], bf, tag="xnm", name=f"xnm_{l}_{grp}")
                    nc.scalar.activation(xnm[:], tp[0:100, :], AF.Copy)
                    xnm_tiles.append(xnm)

                # --- per-chunk: aggregation matmuls + dense matmuls + spill ---
                for ch in range(NCH):
                    # groups needed by this chunk's pairs (+1 group of lookahead)
                    need = min((5 * ch + 4) // 4 + 1, NPAIR // 4 - 1)
                    while len(xnm_tiles) <= need:
                        emit_group(len(xnm_tiles))
                    Ap_t = APP.tile([100, 750], bf, tag="ap", name=f"ap_{l}_{ch}")
                    nc.sync.dma_start(Ap_t[:], Ap_in[ch, :, :])
                    agg_ps = PS.tile([D, CH], f32, tag="agg", bufs=3, name=f"agg_ps_{l}_{ch}")
                    for k in range(5):
                        p = ch * 5 + k
                        g = p // 25
                        xt = xnm_tiles[p // 4]
                        c0 = (p % 4) * 128
                        lhsT = xt[0:100, c0:c0 + 128]
                        nc.tensor.matmul(
                            agg_ps[:, k * 100:(k + 1) * 100],
                            lhsT,
                            Ap_t[0:100, k * 150:k * 150 + 100],
                            start=True, stop=True,
                        )
                        nc.tensor.matmul(
                            xsum_ps[:, g * 50:(g + 1) * 50],
                            lhsT,
                            Ap_t[0:100, k * 150 + 100:k * 150 + 150],
                            start=(p % 25 == 0), stop=(p % 25 == 24),
                            skip_group_check=True,
                        )
                    agg_sb = AGP.tile([D, CH], f32r, tag="aggsb", name=f"agg_sb_{l}_{ch}")
                    nc.vector.tensor_copy(agg_sb[:], agg_ps[:])
                    h_ps = PS.tile([D, CH], f32, tag="h", name=f"h_ps_{l}_{ch}")
                    nc.tensor.matmul(h_ps[:], WrT,
                                     xc[:, ch * CH:(ch + 1) * CH],
                                     start=True, stop=False)
                    nc.tensor.matmul(h_ps[:], WnT, agg_sb[:],
                                     start=False, stop=True)
                    # spill h (raw, bias folded later) + per-chunk sum(h)
                    nc.scalar.activation(xn[:, ch * CH:(ch + 1) * CH], h_ps[:],
                                         AF.Copy, accum_out=accA[:, ch:ch + 1])
                    # sum(h^2): DVE from spilled SBUF copy
                    sqt = SQP.tile([D, CH], f32, tag="sqch", name=f"sq_{l}_{ch}")
                    nc.vector.scalar_tensor_tensor(
                        sqt[:], xn[:, ch * CH:(ch + 1) * CH], 1.0,
                        xn[:, ch * CH:(ch + 1) * CH], OP.mult, OP.mult,
                        accum_out=accB[:, ch:ch + 1])

                # --- h2 branch (original graphs, 400 nodes) ---
                nc.scalar.activation(xsum_f[:], xsum_ps[:], AF.Copy)
                nc.vector.tensor_copy(xsum_b[:], xsum_ps[:])
                tp2 = PS.tile([N, GPC * D], bf, tag="tr", name=f"tp2_{l}")
                for g in range(GPC):
                    nc.tensor.transpose(tp2[0:N, g * D:(g + 1) * D],
                                        xsum_b[:, g * N:(g + 1) * N], identbf[:])
                nc.scalar.activation(xsum_nm[:], tp2[0:N, :], AF.Copy)
                aggo_ps = PS.tile([D, PO], f32, tag="agg", bufs=3, name=f"aggo_ps_{l}")
                for g in range(GPC):
                    nc.tensor.matmul(
                        aggo_ps[:, g * N:(g + 1) * N],
                        xsum_nm[0:N, g * D:(g + 1) * D],
                        Ao_sb[0:N, g * N:(g + 1) * N],
                        start=True, stop=True,
                    )
                nc.vector.tensor_copy(aggo_f[:], aggo_ps[:])
                h2_ps = PS.tile([D, PO], f32, tag="h", name=f"h2_ps_{l}")
                nc.tensor.matmul(h2_ps[:], WsT, xsum_f[:],
                                 start=True, stop=False)
                nc.tensor.matmul(h2_ps[:], WsnT, aggo_f[:],
                                 start=False, stop=True)
                nc.scalar.activation(h2raw[:], h2_ps[:], AF.Copy,
                                     accum_out=stats[:, 2:3])
                nc.vector.scalar_tensor_tensor(
                    sq2[:], h2raw[:], 1.0, h2raw[:], OP.mult, OP.mult,
                    accum_out=stats[:, 3:4])

                # --- pack stats + AllReduce ---
                nc.vector.tensor_reduce(stats[:, 0:1], accA[:], axis=mybir.AxisListType.X,
                                        op=OP.add)
                nc.vector.tensor_reduce(stats[:, 1:2], accB[:], axis=mybir.AxisListType.X,
                                        op=OP.add)
                cc_i = DP.tile([D, 4], f32, tag="cci", name=f"cci_{l}")
                cc_o = DP.tile([D, 4], f32, tag="cco", name=f"cco_{l}")
                nc.gpsimd.dma_start(cc_i[:], stats[:])
                if cc:
                    nc.gpsimd.collective_compute(
                        "AllReduce", OP.add,
                        replica_groups=[list(range(NC))],
                        ins=[cc_i.opt()], outs=[cc_o.opt()],
                    )
                else:
                    nc.gpsimd.dma_start(cc_o[:], cc_i[:])
                nc.gpsimd.dma_start(gstats[:], cc_o[:])

                # --- finalize BN affine params (all [D,1] or [D,2] ops) ---
                # columns of sc: 0=m1 1=m2 2=q1 3=q2 4=var 5=sd 6=inv 7=s1 8=s2
                #                9=mu1 10=mu2 11=t1 12=t2 13..15 tmp
                m = sc[:, 0:2]
                q = sc[:, 2:4]
                nc.vector.tensor_scalar(sc[:, 0:1], gstats[:, 0:1], 1.0 / TN, None, OP.mult)
                nc.vector.tensor_scalar(sc[:, 1:2], gstats[:, 2:3], 1.0 / N_ORIG, None, OP.mult)
                nc.vector.tensor_scalar(sc[:, 2:3], gstats[:, 1:2], 1.0 / TN, None, OP.mult)
                nc.vector.tensor_scalar(sc[:, 3:4], gstats[:, 3:4], 1.0 / N_ORIG, None, OP.mult)
                # var = q - m*m  (then +EPS)
                nc.vector.scalar_tensor_tensor(sc[:, 4:6], m, 1.0, m, OP.mult, OP.mult)
                nc.vector.tensor_tensor(sc[:, 4:6], q, sc[:, 4:6], OP.subtract)
                nc.vector.tensor_scalar(sc[:, 4:6], sc[:, 4:6], EPS, None, OP.add)
                nc.scalar.activation(sc[:, 6:8], sc[:, 4:6], AF.Sqrt)
                nc.vector.reciprocal(sc[:, 8:10], sc[:, 6:8])
                # s1 = bng*inv1 ; s2 = bnsg*inv2
                nc.vector.tensor_tensor(sc[:, 10:11], bng_c, sc[:, 8:9], OP.mult)
                nc.vector.tensor_tensor(sc[:, 11:12], bnsg_c, sc[:, 9:10], OP.mult)
                s1c, s2c = sc[:, 10:11], sc[:, 11:12]
                # mu1 = m1 + b ; mu2 = m2 + bs
                nc.vector.tensor_tensor(sc[:, 12:13], sc[:, 0:1], b_c, OP.add)
                nc.vector.tensor_tensor(sc[:, 13:14], sc[:, 1:2], bs_c, OP.add)
                # t1 = bnb - mu1*s1 ; t2 = bnsb - mu2*s2
                nc.vector.tensor_scalar(sc[:, 12:13], sc[:, 12:13], s1c, None, OP.mult)
                nc.vector.tensor_scalar(sc[:, 13:14], sc[:, 13:14], s2c, None, OP.mult)
                nc.vector.tensor_tensor(sc[:, 12:13], bnb_c, sc[:, 12:13], OP.subtract)
                nc.vector.tensor_tensor(sc[:, 13:14], bnsb_c, sc[:, 13:14], OP.subtract)
                t1c, t2c = sc[:, 12:13], sc[:, 13:14]
                # c2 = s2*bs + t2 + s1*b + t1
                nc.vector.tensor_scalar(sc[:, 14:15], bs_c, s2c, None, OP.mult)
                nc.vector.tensor_tensor(sc[:, 14:15], sc[:, 14:15], t2c, OP.add)
                nc.vector.tensor_scalar(sc[:, 15:16], b_c, s1c, None, OP.mult)
                nc.vector.tensor_tensor(sc[:, 14:15], sc[:, 14:15], sc[:, 15:16], OP.add)
                nc.vector.tensor_tensor(sc[:, 14:15], sc[:, 14:15], t1c, OP.add)
                c2c = sc[:, 14:15]
                # H2 = s2*h2raw + c2
                nc.scalar.activation(H2[:], h2raw[:], AF.Identity, bias=c2c, scale=s2c)

                # --- apply: x_new = relu(s1*h + H2[broadcast]), in-place
                #     per-graph (fewer DVE ops, no scratch) ---
                for g in range(GPC):
                    seg = xn[:, g * GN:(g + 1) * GN]
                    h2b = H2[:, g * N:(g + 1) * N] \
                        .rearrange("p (r n) -> p r n", r=1) \
                        .broadcast_to([D, S, N])
                    nc.vector.scalar_tensor_tensor(
                        seg, seg, s1c, h2b, OP.mult, OP.add)
                    if l == L - 1:
                        nc.vector.tensor_scalar(
                            seg, seg, 0.0, 0.0, OP.max, OP.add,
                            accum_out=accP[:, g:g + 1])
                    else:
                        nc.vector.tensor_scalar(seg, seg, 0.0, None, OP.max)

            # ---- final pooling (means folded into fW1) + MLP ----
            hgr = PP.tile([D, GPC], f32r)
            nc.vector.tensor_copy(hgr[:], accP[:, 0:GPC])
            hg_r = hgr[:]
            o1a = PS.tile([D, GPC], f32, tag="tr", name="o1a")
            o1b = PS.tile([D, GPC], f32, tag="agg", bufs=3, name="o1b")
            nc.tensor.matmul(o1a[:], fW1[:, 0:128], hg_r,
                             start=True, stop=True)
            nc.tensor.matmul(o1b[:], fW1[:, 128:256], hg_r,
                             start=True, stop=True)
            nc.scalar.activation(r1[:, 0:GPC], o1a[:], AF.Relu, bias=fb1[:, 0:1])
            nc.scalar.activation(r1[:, GPC:2 * GPC], o1b[:], AF.Relu, bias=fb1[:, 1:2])
            y_ps = PS.tile([10, GPC], f32, tag="h", name="y_ps")
            nc.tensor.matmul(y_ps[:], fW2[:, 0:10],
                             r1[:, 0:GPC], start=True, stop=False)
            nc.tensor.matmul(y_ps[:], fW2[:, 10:20],
                             r1[:, GPC:2 * GPC], start=False, stop=True)
            nc.scalar.activation(y_sb[:], y_ps[:], AF.Identity, bias=fb2[0:10, 0:1])
            nc.sync.dma_start(Y_out[:], y_sb[:])

    nc_.compile()
    return nc_


def _get_program():
    if "nc" not in _BUILD_CACHE:
        _BUILD_CACHE["nc"] = _build_program()
    return _BUILD_CACHE["nc"]


def _check_structure(edge_index, original_edge_index, batch, subgraph_batch,
                     subgraph_node_idx, num_subgraphs, num_nodes_per_subgraph,
                     subgraph_idx_batch):
    """Verify the node/subgraph bookkeeping matches the layout this kernel
    hardcodes. Returns True when the fast device path is valid."""
    try:
        if not np.array_equal(batch, np.repeat(np.arange(G), S * N)):
            return False
        if not np.array_equal(subgraph_batch, np.tile(np.repeat(np.arange(S), N), G)):
            return False
        if not np.array_equal(subgraph_node_idx, np.tile(np.arange(N), G * S)):
            return False
        if not (np.all(num_subgraphs == S) and np.all(num_nodes_per_subgraph == N)):
            return False
        if not np.array_equal(subgraph_idx_batch, np.repeat(np.arange(G), S)):
            return False
        src, dst = edge_index[0], edge_index[1]
        if not np.array_equal(src // N, dst // N):
            return False
        osrc, odst = original_edge_index[0], original_edge_index[1]
        if not np.array_equal(osrc // N, odst // N):
            return False
        return True
    except Exception:
        return False


def _host_fallback(x, W_root, W_neigh, b, bng, bnb, Ws_root, Ws_neigh, bs, bnsg,
                   bnsb, fW1, fb1, fW2, fb2, edge_index, original_edge_index,
                   batch, subgraph_batch, subgraph_node_idx, num_subgraphs,
                   num_nodes_per_subgraph, subgraph_idx_batch):
    """Pure numpy replica of the reference, used only if the structural
    assumptions of the device path do not hold."""
    def seg_sum(v, idx, n):
        out = np.zeros((n, v.shape[1]), v.dtype)
        np.add.at(out, idx, v)
        return out

    def seg_mean(v, idx, n):
        s = seg_sum(v, idx, n)
        c = np.zeros((n, 1), v.dtype)
        np.add.at(c, idx, np.ones((v.shape[0], 1), v.dtype))
        return s / np.maximum(c, 1.0)

    def bnorm(h, gamma, beta):
        mu = h.mean(0)
        var = ((h - mu) ** 2).mean(0)
        return gamma * (h - mu) / np.sqrt(var + EPS) + beta

    def conv(v, s_, d_, Wr_, Wn_, b_, n):
        agg = seg_sum(v[s_], d_, n)
        return v @ Wr_ + agg @ Wn_ + b_

    x = x.astype(np.float64)
    node_off = np.concatenate([[0], np.cumsum(num_nodes_per_subgraph)])
    node_idx = node_off[batch] + subgraph_node_idx
    src, dst = edge_index[0], edge_index[1]
    osrc, odst = original_edge_index[0], original_edge_index[1]
    for i in range(L):
        h1 = bnorm(conv(x, src, dst, W_root[i], W_neigh[i], b[i], TN), bng[i], bnb[i])
        x_sum = seg_mean(x, node_idx, N_ORIG)
        h2 = bnorm(conv(x_sum, osrc, odst, Ws_root[i], Ws_neigh[i], bs[i], N_ORIG),
                   bnsg[i], bnsb[i])
        x = np.maximum(h1 + h2[node_idx], 0.0)
    sub_off = np.concatenate([[0], np.cumsum(num_subgraphs)])
    subgraph_idx = sub_off[batch] + subgraph_batch
    h_sub = seg_mean(x, subgraph_idx, N_SUBG)
    h_g = seg_mean(h_sub, subgraph_idx_batch, G)
    return (np.maximum(h_g @ fW1 + fb1, 0.0) @ fW2 + fb2).astype(np.float32)


def kernel(**inputs) -> np.ndarray:
    from concourse.bass_utils import run_bass_kernel_spmd

    inp = {k: np.asarray(v) for k, v in inputs.items()}
    x = inp["x"].astype(np.float32)

    if not _check_structure(
            inp["edge_index"], inp["original_edge_index"], inp["batch"],
            inp["subgraph_batch"], inp["subgraph_node_idx"], inp["num_subgraphs"],
            inp["num_nodes_per_subgraph"], inp["subgraph_idx_batch"]):
        return _host_fallback(**{k: np.asarray(v, np.float64)
                                 if np.asarray(v).dtype.kind == "f" else np.asarray(v)
                                 for k, v in inp.items()})

    in_maps = _make_in_maps(inp, x)
    nc = _get_program()
    res = run_bass_kernel_spmd(nc, in_maps, core_ids=list(range(NC)))
    return _assemble(res.results)


def _make_in_maps(inp, x):
    # per-subgraph adjacency counts: A[sub, ls, ld] = #edges(src -> dst)
    src = inp["edge_index"][0].astype(np.int64)
    dst = inp["edge_index"][1].astype(np.int64)
    sub = src // N
    A = np.bincount((sub * N + src % N) * N + dst % N,
                    minlength=N_SUBG * N * N).reshape(N_SUBG, N, N)
    osrc = inp["original_edge_index"][0].astype(np.int64)
    odst = inp["original_edge_index"][1].astype(np.int64)
    og = osrc // N
    Ao = np.bincount((og * N + osrc % N) * N + odst % N,
                     minlength=G * N * N).reshape(G, N, N)

    # pair blocks [100, 150]: block-diag adjacency + identity columns
    Ap = A.reshape(NC, NPAIR, 2, N, N).astype(np.float32)
    arr = np.zeros((NC, NPAIR, 100, 150), np.float32)
    arr[:, :, 0:N, 0:N] = Ap[:, :, 0]
    arr[:, :, N:100, N:100] = Ap[:, :, 1]
    eye = np.eye(N, dtype=np.float32)
    arr[:, :, 0:N, 100:150] = eye
    arr[:, :, N:100, 100:150] = eye
    # [NC, 100, NPAIR*150] -> chunk-major [NC, NCH, 100, 750] (contiguous DMAs)
    Apair = np.ascontiguousarray(
        arr.transpose(0, 2, 1, 3).reshape(NC, 100, NCH, 750)
        .transpose(0, 2, 1, 3)).astype(BF16)

    AoT = np.ascontiguousarray(
        Ao.reshape(NC, GPC, N, N).astype(np.float32)
        .transpose(0, 2, 1, 3).reshape(NC, N, PO)).astype(BF16)

    xT = np.ascontiguousarray(
        x.reshape(NC, PN, D).transpose(0, 2, 1)
        .reshape(NC, D, 10, PN // 10).transpose(0, 2, 1, 3)).astype(np.float32)

    def wpack(w, scale=1.0):
        # [L, D, D] -> [D, L*D] with lhsT = W[l] (contract dim on partitions)
        return np.ascontiguousarray(
            (np.asarray(w, np.float32) * scale).transpose(1, 0, 2).reshape(D, L * D))

    Wr_h = wpack(inp["W_root"])
    Wn_h = wpack(inp["W_neigh"])
    Ws_h = wpack(inp["Ws_root"], 1.0 / S)
    Wsn_h = wpack(inp["Ws_neigh"], 1.0 / S)

    vecs = np.zeros((D, L * 6), np.float32)
    for l in range(L):
        vecs[:, l * 6 + 0] = inp["bng"][l]
        vecs[:, l * 6 + 1] = inp["bnb"][l]
        vecs[:, l * 6 + 2] = inp["bnsg"][l]
        vecs[:, l * 6 + 3] = inp["bnsb"][l]
        vecs[:, l * 6 + 4] = inp["b"][l]
        vecs[:, l * 6 + 5] = inp["bs"][l]

    fW1_h = np.ascontiguousarray(inp["fW1"].astype(np.float32) / GN)
    fW2_h = np.zeros((D, 20), np.float32)
    fW2_h[:, 0:10] = inp["fW2"][0:128]
    fW2_h[:, 10:20] = inp["fW2"][128:256]
    fb1_h = np.ascontiguousarray(inp["fb1"].astype(np.float32).reshape(2, 128).T)
    fb2_h = inp["fb2"].astype(np.float32).reshape(10, 1)
    ident = np.eye(D, dtype=np.float32)
    identbf = np.eye(D, dtype=np.float32).astype(BF16)

    in_maps = []
    for c in range(NC):
        in_maps.append({
            "xT": xT[c],
            "Apair": Apair[c],
            "AoT": AoT[c],
            "Wr": Wr_h, "Wn": Wn_h, "Ws": Ws_h, "Wsn": Wsn_h,
            "vecs": vecs,
            "fW1": fW1_h, "fW2": fW2_h, "fb1": fb1_h, "fb2": fb2_h,
            "ident": ident, "identbf": identbf,
        })
    return in_maps


def _assemble(results):
    out = np.zeros((G, 10), np.float32)
    for c in range(NC):
        out[c * GPC:(c + 1) * GPC] = results[c]["Y"].T
    return out


# revision 31
# speedup vs baseline: 1.0684x; 1.0684x over previous
"""Trainium2 Bass kernel for nn_DSSnetwork (DSS-GNN message passing).

Strategy (graph-level data parallelism, 8 graphs per core):
  - x kept SBUF-resident, feature-major [128, 20000] fp32, ping-pong across layers.
  - Per-subgraph aggregation as dense block-diagonal pair matmuls on PE
    (stationary = transposed node-major x pair [100,128] bf16, moving = pair
    adjacency [100,100] bf16 streamed from HBM; extra 50 identity columns
    accumulate the cross-subgraph-copy sum for the h2 branch).
  - Dense W matmuls in float32r (fp22 multiplies) at 1 cycle/row.
  - BatchNorm is global across all 160000 (resp 3200) rows: per-core
    (sum, sumsq) go through one 2KB AllReduce per layer; affine apply is
    folded into per-feature scale/bias vectors.
"""

import numpy as np
import ml_dtypes

# ---- static problem sizes (must match reference.setup_inputs) ----
G, S, N, D, DEG, L = 64, 50, 50, 128, 8, 4
TN = G * S * N
N_ORIG = G * N
N_SUBG = G * S
EPS = 1e-5

NC = 8                 # cores
GPC = G // NC          # graphs per core = 8
PN = GPC * S * N       # nodes per core = 20000
PO = GPC * N           # original nodes per core = 400
NPAIR = GPC * S // 2   # subgraph pairs per core = 200
CH = 500               # node chunk (5 pairs, 10 subgraphs)
NCH = PN // CH         # 40 chunks
GN = S * N             # nodes per graph = 2500
HG = GN // 2           # half graph = 1250
PNP = 20096            # PN padded to multiple of 128 (157 tiles)

BF16 = ml_dtypes.bfloat16

_BUILD_CACHE = {}


def _build_program(cc=True):
    import concourse.bass as bass
    import concourse.mybir as mybir
    import concourse.tile as tile
    import concourse.bacc as bacc

    f32 = mybir.dt.float32
    f32r = mybir.dt.float32r
    bf = mybir.dt.bfloat16
    AF = mybir.ActivationFunctionType
    OP = mybir.AluOpType

    nc_ = bacc.Bacc("TRN2", target_bir_lowering=False, debug=False, num_devices=NC)

    with tile.TileContext(nc_) as tc:
        nc = tc.nc

        xT_in = nc.dram_tensor("xT", [10, D, PN // 10], f32r, kind="ExternalInput")
        Ap_in = nc.dram_tensor("Apair", [NCH, 100, 750], bf, kind="ExternalInput")
        Ao_in = nc.dram_tensor("AoT", [N, PO], bf, kind="ExternalInput")
        Wr_in = nc.dram_tensor("Wr", [D, L * D], f32r, kind="ExternalInput")
        Wn_in = nc.dram_tensor("Wn", [D, L * D], f32r, kind="ExternalInput")
        Ws_in = nc.dram_tensor("Ws", [D, L * D], f32r, kind="ExternalInput")
        Wsn_in = nc.dram_tensor("Wsn", [D, L * D], f32r, kind="ExternalInput")
        vecs_in = nc.dram_tensor("vecs", [D, L * 6], f32, kind="ExternalInput")
        fW1_in = nc.dram_tensor("fW1", [D, 256], f32r, kind="ExternalInput")
        fW2_in = nc.dram_tensor("fW2", [D, 20], f32r, kind="ExternalInput")
        fb1_in = nc.dram_tensor("fb1", [D, 2], f32, kind="ExternalInput")
        fb2_in = nc.dram_tensor("fb2", [10, 1], f32, kind="ExternalInput")
        id_in = nc.dram_tensor("ident", [D, D], f32r, kind="ExternalInput")
        idb_in = nc.dram_tensor("identbf", [D, D], bf, kind="ExternalInput")
        Y_out = nc.dram_tensor("Y", [10, GPC], f32, kind="ExternalOutput")

        with (
            tc.tile_pool(name="persist", bufs=1) as PP,
            tc.tile_pool(name="xnmp", bufs=6) as XP,
            tc.tile_pool(name="aggp", bufs=3) as AGP,
            tc.tile_pool(name="app", bufs=3) as APP,
            tc.tile_pool(name="castp", bufs=5) as CSP,
            tc.tile_pool(name="sqp", bufs=2) as SQP,
            tc.tile_pool(name="ps", bufs=2, space="PSUM") as PS,
            tc.tile_pool(name="ps1", bufs=1, space="PSUM") as PS1,
            tc.tile_pool(name="dram", bufs=2, space="DRAM") as DP,
        ):
            xa = PP.tile([D, PN], f32r)
            xb = PP.tile([D, PN], f32r)
            Wr = PP.tile([D, L * D], f32r)
            Wn = PP.tile([D, L * D], f32r)
            Ws = PP.tile([D, L * D], f32r)
            Wsn = PP.tile([D, L * D], f32r)
            vecs = PP.tile([D, L * 6], f32)
            fW1 = PP.tile([D, 256], f32r)
            fW2 = PP.tile([D, 20], f32r)
            fb1 = PP.tile([D, 2], f32)
            fb2 = PP.tile([10, 1], f32)
            ident = PP.tile([D, D], f32r)
            identbf = PP.tile([D, D], bf)
            Ao_sb = PP.tile([N, PO], bf)
            accA = PP.tile([D, NCH], f32)
            accB = PP.tile([D, NCH], f32)
            accP = PP.tile([D, NCH], f32)
            hg = PP.tile([D, GPC], f32)
            stats = PP.tile([D, 4], f32)
            gstats = PP.tile([D, 4], f32)
            xsum_f = PP.tile([D, PO], f32r)
            xsum_b = PP.tile([D, PO], bf)
            xsum_nm = PP.tile([N, GPC * D], bf)
            aggo_f = PP.tile([D, PO], f32r)
            h2raw = PP.tile([D, PO], f32)
            H2 = PP.tile([D, PO], f32)
            sq2 = PP.tile([D, PO], f32)
            sc = PP.tile([D, 16], f32)
            r1 = PP.tile([D, 16], f32r)
            y_sb = PP.tile([10, GPC], f32)

            # ---- loads ----
            for i in range(10):
                nc.sync.dma_start(xa[:, i * 2000:(i + 1) * 2000],
                                  xT_in[i, :, :])
            nc.sync.dma_start(Wr[:], Wr_in[:])
            nc.sync.dma_start(Wn[:], Wn_in[:])
            nc.sync.dma_start(Ws[:], Ws_in[:])
            nc.sync.dma_start(Wsn[:], Wsn_in[:])
            nc.sync.dma_start(vecs[:], vecs_in[:])
            nc.sync.dma_start(fW1[:], fW1_in[:])
            nc.sync.dma_start(fW2[:], fW2_in[:])
            nc.sync.dma_start(fb1[:], fb1_in[:])
            nc.sync.dma_start(fb2[:], fb2_in[:])
            nc.sync.dma_start(ident[:], id_in[:])
            nc.sync.dma_start(identbf[:], idb_in[:])
            nc.sync.dma_start(Ao_sb[:], Ao_in[:])

            xbufs = [xa, xb]
            for l in range(L):
                xc = xbufs[l % 2]
                xn = xbufs[(l + 1) % 2]
                WrT = Wr[:, l * D:(l + 1) * D]
                WnT = Wn[:, l * D:(l + 1) * D]
                WsT = Ws[:, l * D:(l + 1) * D]
                WsnT = Wsn[:, l * D:(l + 1) * D]
                bng_c = vecs[:, l * 6 + 0:l * 6 + 1]
                bnb_c = vecs[:, l * 6 + 1:l * 6 + 2]
                bnsg_c = vecs[:, l * 6 + 2:l * 6 + 3]
                bnsb_c = vecs[:, l * 6 + 3:l * 6 + 4]
                b_c = vecs[:, l * 6 + 4:l * 6 + 5]
                bs_c = vecs[:, l * 6 + 5:l * 6 + 6]

                xsum_ps = PS1.tile([D, PO], f32, tag="xsum", name=f"xsum_ps_{l}")

                # --- transposes are emitted lazily, interleaved with chunks,
                #     so the PE stream pipelines instead of phase-serializing ---
                xnm_tiles = []

                def emit_group(grp):
                    tp = PS.tile([100, 512], f32r, tag="tr", name=f"tp_{l}_{grp}")
                    for j in range(4):
                        p = grp * 4 + j
                        nc.tensor.transpose(
                            tp[0:100, j * 128:(j + 1) * 128],
                            xc[:, p * 100:(p + 1) * 100],
                            ident[:],
                        )
                    xnm = XP.tile([100, 512], bf, tag="xnm", name=f"xnm_{l}_{grp}")
                    nc.scalar.activation(xnm[:], tp[0:100, :], AF.Copy)
                    xnm_tiles.append(xnm)

                # --- per-chunk: aggregation matmuls + dense matmuls + spill ---
                for ch in range(NCH):
                    # groups needed by this chunk's pairs (+1 group of lookahead)
                    need = min((5 * ch + 4) // 4 + 1, NPAIR // 4 - 1)
                    while len(xnm_tiles) <= need:
                        emit_group(len(xnm_tiles))
                    Ap_t = APP.tile([100, 750], bf, tag="ap", name=f"ap_{l}_{ch}")
                    nc.sync.dma_start(Ap_t[:], Ap_in[ch, :, :])
                    agg_ps = PS.tile([D, CH], f32, tag="agg", bufs=3, name=f"agg_ps_{l}_{ch}")
                    for k in range(5):
                        p = ch * 5 + k
                        g = p // 25
                        xt = xnm_tiles[p // 4]
                        c0 = (p % 4) * 128
                        lhsT = xt[0:100, c0:c0 + 128]
                        nc.tensor.matmul(
                            agg_ps[:, k * 100:(k + 1) * 100],
                            lhsT,
                            Ap_t[0:100, k * 150:k * 150 + 100],
                            start=True, stop=True,
                        )
                        nc.tensor.matmul(
                            xsum_ps[:, g * 50:(g + 1) * 50],
                            lhsT,
                            Ap_t[0:100, k * 150 + 100:k * 150 + 150],
                            start=(p % 25 == 0), stop=(p % 25 == 24),
                            skip_group_check=True,
                        )
                    agg_sb = AGP.tile([D, CH], f32r, tag="aggsb", name=f"agg_sb_{l}_{ch}")
                    nc.vector.tensor_copy(agg_sb[:], agg_ps[:])
                    h_ps = PS.tile([D, CH], f32, tag="h", name=f"h_ps_{l}_{ch}")
                    nc.tensor.matmul(h_ps[:], WrT,
                                     xc[:, ch * CH:(ch + 1) * CH],
                                     start=True, stop=False)
                    nc.tensor.matmul(h_ps[:], WnT, agg_sb[:],
                                     start=False, stop=True)
                    # spill h (raw, bias folded later) + per-chunk sum(h)
                    nc.scalar.activation(xn[:, ch * CH:(ch + 1) * CH], h_ps[:],
                                         AF.Copy, accum_out=accA[:, ch:ch + 1])
                    # sum(h^2): DVE from spilled SBUF copy
                    sqt = SQP.tile([D, CH], f32, tag="sqch", name=f"sq_{l}_{ch}")
                    nc.vector.scalar_tensor_tensor(
                        sqt[:], xn[:, ch * CH:(ch + 1) * CH], 1.0,
                        xn[:, ch * CH:(ch + 1) * CH], OP.mult, OP.mult,
                        accum_out=accB[:, ch:ch + 1])

                # --- h2 branch (original graphs, 400 nodes) ---
                nc.scalar.activation(xsum_f[:], xsum_ps[:], AF.Copy)
                nc.vector.tensor_copy(xsum_b[:], xsum_ps[:])
                tp2 = PS.tile([N, GPC * D], bf, tag="tr", name=f"tp2_{l}")
                for g in range(GPC):
                    nc.tensor.transpose(tp2[0:N, g * D:(g + 1) * D],
                                        xsum_b[:, g * N:(g + 1) * N], identbf[:])
                nc.scalar.activation(xsum_nm[:], tp2[0:N, :], AF.Copy)
                aggo_ps = PS.tile([D, PO], f32, tag="agg", bufs=3, name=f"aggo_ps_{l}")
                for g in range(GPC):
                    nc.tensor.matmul(
                        aggo_ps[:, g * N:(g + 1) * N],
                        xsum_nm[0:N, g * D:(g + 1) * D],
                        Ao_sb[0:N, g * N:(g + 1) * N],
                        start=True, stop=True,
                    )
                nc.vector.tensor_copy(aggo_f[:], aggo_ps[:])
                h2_ps = PS.tile([D, PO], f32, tag="h", name=f"h2_ps_{l}")
                nc.tensor.matmul(h2_ps[:], WsT, xsum_f[:],
                                 start=True, stop=False)
                nc.tensor.matmul(h2_ps[:], WsnT, aggo_f[:],
                                 start=False, stop=True)
                nc.scalar.activation(h2raw[:], h2_ps[:], AF.Copy,
                                     accum_out=stats[:, 2:3])
                nc.vector.scalar_tensor_tensor(
                    sq2[:], h2raw[:], 1.0, h2raw[:], OP.mult, OP.mult,
                    accum_out=stats[:, 3:4])

                # --- pack stats + AllReduce ---
                nc.vector.tensor_reduce(stats[:, 0:1], accA[:], axis=mybir.AxisListType.X,
                                        op=OP.add)
                nc.vector.tensor_reduce(stats[:, 1:2], accB[:], axis=mybir.AxisListType.X,
                                        op=OP.add)
                cc_i = DP.tile([D, 4], f32, tag="cci", name=f"cci_{l}")
                cc_o = DP.tile([D, 4], f32, tag="cco", name=f"cco_{l}")
                nc.gpsimd.dma_start(cc_i[:], stats[:])
                if cc:
                    nc.gpsimd.collective_compute(
                        "AllReduce", OP.add,
                        replica_groups=[list(range(NC))],
                        ins=[cc_i.opt()], outs=[cc_o.opt()],
                    )
                else:
                    nc.gpsimd.dma_start(cc_o[:], cc_i[:])
                nc.gpsimd.dma_start(gstats[:], cc_o[:])

                # --- finalize BN affine params (all [D,1] or [D,2] ops) ---
                # columns of sc: 0=m1 1=m2 2=q1 3=q2 4=var 5=sd 6=inv 7=s1 8=s2
                #                9=mu1 10=mu2 11=t1 12=t2 13..15 tmp
                m = sc[:, 0:2]
                q = sc[:, 2:4]
                nc.vector.tensor_scalar(sc[:, 0:1], gstats[:, 0:1], 1.0 / TN, None, OP.mult)
                nc.vector.tensor_scalar(sc[:, 1:2], gstats[:, 2:3], 1.0 / N_ORIG, None, OP.mult)
                nc.vector.tensor_scalar(sc[:, 2:3], gstats[:, 1:2], 1.0 / TN, None, OP.mult)
                nc.vector.tensor_scalar(sc[:, 3:4], gstats[:, 3:4], 1.0 / N_ORIG, None, OP.mult)
                # var = q - m*m  (then +EPS)
                nc.vector.scalar_tensor_tensor(sc[:, 4:6], m, 1.0, m, OP.mult, OP.mult)
                nc.vector.tensor_tensor(sc[:, 4:6], q, sc[:, 4:6], OP.subtract)
                nc.vector.tensor_scalar(sc[:, 4:6], sc[:, 4:6], EPS, None, OP.add)
                nc.scalar.activation(sc[:, 6:8], sc[:, 4:6], AF.Sqrt)
                nc.vector.reciprocal(sc[:, 8:10], sc[:, 6:8])
                # s1 = bng*inv1 ; s2 = bnsg*inv2
                nc.vector.tensor_tensor(sc[:, 10:11], bng_c, sc[:, 8:9], OP.mult)
                nc.vector.tensor_tensor(sc[:, 11:12], bnsg_c, sc[:, 9:10], OP.mult)
                s1c, s2c = sc[:, 10:11], sc[:, 11:12]
                # mu1 = m1 + b ; mu2 = m2 + bs
                nc.vector.tensor_tensor(sc[:, 12:13], sc[:, 0:1], b_c, OP.add)
                nc.vector.tensor_tensor(sc[:, 13:14], sc[:, 1:2], bs_c, OP.add)
                # t1 = bnb - mu1*s1 ; t2 = bnsb - mu2*s2
                nc.vector.tensor_scalar(sc[:, 12:13], sc[:, 12:13], s1c, None, OP.mult)
                nc.vector.tensor_scalar(sc[:, 13:14], sc[:, 13:14], s2c, None, OP.mult)
                nc.vector.tensor_tensor(sc[:, 12:13], bnb_c, sc[:, 12:13], OP.subtract)
                nc.vector.tensor_tensor(sc[:, 13:14], bnsb_c, sc[:, 13:14], OP.subtract)
                t1c, t2c = sc[:, 12:13], sc[:, 13:14]
                # c2 = s2*bs + t2 + s1*b + t1
                nc.vector.tensor_scalar(sc[:, 14:15], bs_c, s2c, None, OP.mult)
                nc.vector.tensor_tensor(sc[:, 14:15], sc[:, 14:15], t2c, OP.add)
                nc.vector.tensor_scalar(sc[:, 15:16], b_c, s1c, None, OP.mult)
                nc.vector.tensor_tensor(sc[:, 14:15], sc[:, 14:15], sc[:, 15:16], OP.add)
                nc.vector.tensor_tensor(sc[:, 14:15], sc[:, 14:15], t1c, OP.add)
                c2c = sc[:, 14:15]
                # H2 = s2*h2raw + c2
                nc.scalar.activation(H2[:], h2raw[:], AF.Identity, bias=c2c, scale=s2c)

                # --- apply: x_new = relu(s1*h + H2[broadcast]), in-place
                #     per-graph (fewer DVE ops, no scratch) ---
                for g in range(GPC):
                    seg = xn[:, g * GN:(g + 1) * GN]
                    h2b = H2[:, g * N:(g + 1) * N] \
                        .rearrange("p (r n) -> p r n", r=1) \
                        .broadcast_to([D, S, N])
                    nc.vector.scalar_tensor_tensor(
                        seg, seg, s1c, h2b, OP.mult, OP.add)
                    # relu on ACT: pipelines with the next graph's DVE add
                    if l == L - 1:
                        nc.scalar.activation(seg, seg, AF.Relu,
                                             accum_out=accP[:, g:g + 1])
                    else:
                        nc.scalar.activation(seg, seg, AF.Relu)

            # ---- final pooling (means folded into fW1) + MLP ----
            hgr = PP.tile([D, GPC], f32r)
            nc.vector.tensor_copy(hgr[:], accP[:, 0:GPC])
            hg_r = hgr[:]
            o1a = PS.tile([D, GPC], f32, tag="tr", name="o1a")
            o1b = PS.tile([D, GPC], f32, tag="agg", bufs=3, name="o1b")
            nc.tensor.matmul(o1a[:], fW1[:, 0:128], hg_r,
                             start=True, stop=True)
            nc.tensor.matmul(o1b[:], fW1[:, 128:256], hg_r,
                             start=True, stop=True)
            nc.scalar.activation(r1[:, 0:GPC], o1a[:], AF.Relu, bias=fb1[:, 0:1])
            nc.scalar.activation(r1[:, GPC:2 * GPC], o1b[:], AF.Relu, bias=fb1[:, 1:2])
            y_ps = PS.tile([10, GPC], f32, tag="h", name="y_ps")
            nc.tensor.matmul(y_ps[:], fW2[:, 0:10],
                             r1[:, 0:GPC], start=True, stop=False)
            nc.tensor.matmul(y_ps[:], fW2[:, 10:20],
                             r1[:, GPC:2 * GPC], start=False, stop=True)
            nc.scalar.activation(y_sb[:], y_ps[:], AF.Identity, bias=fb2[0:10, 0:1])
            nc.sync.dma_start(Y_out[:], y_sb[:])

    nc_.compile()
    return nc_


def _get_program():
    if "nc" not in _BUILD_CACHE:
        _BUILD_CACHE["nc"] = _build_program()
    return _BUILD_CACHE["nc"]


def _check_structure(edge_index, original_edge_index, batch, subgraph_batch,
                     subgraph_node_idx, num_subgraphs, num_nodes_per_subgraph,
                     subgraph_idx_batch):
    """Verify the node/subgraph bookkeeping matches the layout this kernel
    hardcodes. Returns True when the fast device path is valid."""
    try:
        if not np.array_equal(batch, np.repeat(np.arange(G), S * N)):
            return False
        if not np.array_equal(subgraph_batch, np.tile(np.repeat(np.arange(S), N), G)):
            return False
        if not np.array_equal(subgraph_node_idx, np.tile(np.arange(N), G * S)):
            return False
        if not (np.all(num_subgraphs == S) and np.all(num_nodes_per_subgraph == N)):
            return False
        if not np.array_equal(subgraph_idx_batch, np.repeat(np.arange(G), S)):
            return False
        src, dst = edge_index[0], edge_index[1]
        if not np.array_equal(src // N, dst // N):
            return False
        osrc, odst = original_edge_index[0], original_edge_index[1]
        if not np.array_equal(osrc // N, odst // N):
            return False
        return True
    except Exception:
        return False


def _host_fallback(x, W_root, W_neigh, b, bng, bnb, Ws_root, Ws_neigh, bs, bnsg,
                   bnsb, fW1, fb1, fW2, fb2, edge_index, original_edge_index,
                   batch, subgraph_batch, subgraph_node_idx, num_subgraphs,
                   num_nodes_per_subgraph, subgraph_idx_batch):
    """Pure numpy replica of the reference, used only if the structural
    assumptions of the device path do not hold."""
    def seg_sum(v, idx, n):
        out = np.zeros((n, v.shape[1]), v.dtype)
        np.add.at(out, idx, v)
        return out

    def seg_mean(v, idx, n):
        s = seg_sum(v, idx, n)
        c = np.zeros((n, 1), v.dtype)
        np.add.at(c, idx, np.ones((v.shape[0], 1), v.dtype))
        return s / np.maximum(c, 1.0)

    def bnorm(h, gamma, beta):
        mu = h.mean(0)
        var = ((h - mu) ** 2).mean(0)
        return gamma * (h - mu) / np.sqrt(var + EPS) + beta

    def conv(v, s_, d_, Wr_, Wn_, b_, n):
        agg = seg_sum(v[s_], d_, n)
        return v @ Wr_ + agg @ Wn_ + b_

    x = x.astype(np.float64)
    node_off = np.concatenate([[0], np.cumsum(num_nodes_per_subgraph)])
    node_idx = node_off[batch] + subgraph_node_idx
    src, dst = edge_index[0], edge_index[1]
    osrc, odst = original_edge_index[0], original_edge_index[1]
    for i in range(L):
        h1 = bnorm(conv(x, src, dst, W_root[i], W_neigh[i], b[i], TN), bng[i], bnb[i])
        x_sum = seg_mean(x, node_idx, N_ORIG)
        h2 = bnorm(conv(x_sum, osrc, odst, Ws_root[i], Ws_neigh[i], bs[i], N_ORIG),
                   bnsg[i], bnsb[i])
        x = np.maximum(h1 + h2[node_idx], 0.0)
    sub_off = np.concatenate([[0], np.cumsum(num_subgraphs)])
    subgraph_idx = sub_off[batch] + subgraph_batch
    h_sub = seg_mean(x, subgraph_idx, N_SUBG)
    h_g = seg_mean(h_sub, subgraph_idx_batch, G)
    return (np.maximum(h_g @ fW1 + fb1, 0.0) @ fW2 + fb2).astype(np.float32)


def kernel(**inputs) -> np.ndarray:
    from concourse.bass_utils import run_bass_kernel_spmd

    inp = {k: np.asarray(v) for k, v in inputs.items()}
    x = inp["x"].astype(np.float32)

    if not _check_structure(
            inp["edge_index"], inp["original_edge_index"], inp["batch"],
            inp["subgraph_batch"], inp["subgraph_node_idx"], inp["num_subgraphs"],
            inp["num_nodes_per_subgraph"], inp["subgraph_idx_batch"]):
        return _host_fallback(**{k: np.asarray(v, np.float64)
                                 if np.asarray(v).dtype.kind == "f" else np.asarray(v)
                                 for k, v in inp.items()})

    in_maps = _make_in_maps(inp, x)
    nc = _get_program()
    res = run_bass_kernel_spmd(nc, in_maps, core_ids=list(range(NC)))
    return _assemble(res.results)


def _make_in_maps(inp, x):
    # per-subgraph adjacency counts: A[sub, ls, ld] = #edges(src -> dst)
    src = inp["edge_index"][0].astype(np.int64)
    dst = inp["edge_index"][1].astype(np.int64)
    sub = src // N
    A = np.bincount((sub * N + src % N) * N + dst % N,
                    minlength=N_SUBG * N * N).reshape(N_SUBG, N, N)
    osrc = inp["original_edge_index"][0].astype(np.int64)
    odst = inp["original_edge_index"][1].astype(np.int64)
    og = osrc // N
    Ao = np.bincount((og * N + osrc % N) * N + odst % N,
                     minlength=G * N * N).reshape(G, N, N)

    # pair blocks [100, 150]: block-diag adjacency + identity columns
    Ap = A.reshape(NC, NPAIR, 2, N, N).astype(np.float32)
    arr = np.zeros((NC, NPAIR, 100, 150), np.float32)
    arr[:, :, 0:N, 0:N] = Ap[:, :, 0]
    arr[:, :, N:100, N:100] = Ap[:, :, 1]
    eye = np.eye(N, dtype=np.float32)
    arr[:, :, 0:N, 100:150] = eye
    arr[:, :, N:100, 100:150] = eye
    # [NC, 100, NPAIR*150] -> chunk-major [NC, NCH, 100, 750] (contiguous DMAs)
    Apair = np.ascontiguousarray(
        arr.transpose(0, 2, 1, 3).reshape(NC, 100, NCH, 750)
        .transpose(0, 2, 1, 3)).astype(BF16)

    AoT = np.ascontiguousarray(
        Ao.reshape(NC, GPC, N, N).astype(np.float32)
        .transpose(0, 2, 1, 3).reshape(NC, N, PO)).astype(BF16)

    xT = np.ascontiguousarray(
        x.reshape(NC, PN, D).transpose(0, 2, 1)
        .reshape(NC, D, 10, PN // 10).transpose(0, 2, 1, 3)).astype(np.float32)

    def wpack(w, scale=1.0):
        # [L, D, D] -> [D, L*D] with lhsT = W[l] (contract dim on partitions)
        return np.ascontiguousarray(
            (np.asarray(w, np.float32) * scale).transpose(1, 0, 2).reshape(D, L * D))

    Wr_h = wpack(inp["W_root"])
    Wn_h = wpack(inp["W_neigh"])
    Ws_h = wpack(inp["Ws_root"], 1.0 / S)
    Wsn_h = wpack(inp["Ws_neigh"], 1.0 / S)

    vecs = np.zeros((D, L * 6), np.float32)
    for l in range(L):
        vecs[:, l * 6 + 0] = inp["bng"][l]
        vecs[:, l * 6 + 1] = inp["bnb"][l]
        vecs[:, l * 6 + 2] = inp["bnsg"][l]
        vecs[:, l * 6 + 3] = inp["bnsb"][l]
        vecs[:, l * 6 + 4] = inp["b"][l]
        vecs[:, l * 6 + 5] = inp["bs"][l]

    fW1_h = np.ascontiguousarray(inp["fW1"].astype(np.float32) / GN)
    fW2_h = np.zeros((D, 20), np.float32)
    fW2_h[:, 0:10] = inp["fW2"][0:128]
    fW2_h[:, 10:20] = inp["fW2"][128:256]
    fb1_h = np.ascontiguousarray(inp["fb1"].astype(np.float32).reshape(2, 128).T)
    fb2_h = inp["fb2"].astype(np.float32).reshape(10, 1)
    ident = np.eye(D, dtype=np.float32)
    identbf = np.eye(D, dtype=np.float32).astype(BF16)

    in_maps = []
    for c in range(NC):
        in_maps.append({
            "xT": xT[c],
            "Apair": Apair[c],
            "AoT": AoT[c],
            "Wr": Wr_h, "Wn": Wn_h, "Ws": Ws_h, "Wsn": Wsn_h,
            "vecs": vecs,
            "fW1": fW1_h, "fW2": fW2_h, "fb1": fb1_h, "fb2": fb2_h,
            "ident": ident, "identbf": identbf,
        })
    return in_maps


def _assemble(results):
    out = np.zeros((G, 10), np.float32)
    for c in range(NC):
        out[c * GPC:(c + 1) * GPC] = results[c]["Y"].T
    return out
